# revision 1
# baseline (speedup 1.0000x reference)
"""Conv2D 3x3 (stride 1, pad 1) NCHW kernel for Trainium2, 8 NeuronCores.

Reference op: y = conv2d(x[32,128,56,56], w[256,128,3,3]) + b[256]  (fp32)

Strategy:
  - Data-parallel over batch: 4 images per core, 8 cores.
  - Conv as 9 shifted matmuls accumulating in PSUM:
      out[co, (h,w)] = sum_{kh,kw} W[kh,kw][ci,co].T @ xpad[ci, h+kh, w+kw]
    K = ci = 128 sits exactly on the 128 SBUF partitions.
  - Host pre-pads x to 58x58 (zero halo) and pre-transposes the weight to
    [ci, (kh kw), co], so the device never transposes or memsets anything.
  - fp16 operands (host-cast): full PE rate (1 col/cycle), separate
    LDWEIGHTS with FWL that the PE's reorder window hides behind matmuls.
    Measured full-conv rel err ~= a few 1e-4 (PSUM accumulates fp32).
  - Output tiles are 8 rows x 56 cols = 448 fp32 = one PSUM bank; bias is
    fused into the PSUM->SBUF eviction on the Scalar engine (Identity+bias).
"""

import numpy as np

N_CORES = 8
B, CI, H, W = 32, 128, 56, 56
CO = 256
KH = KW = 3
NTAPS = KH * KW
BS = B // N_CORES            # images per core
HP, WP = H + 2, W + 2        # padded image
HB = 8                       # output rows per block
NB = H // HB                 # blocks per image
NTILE = HB * W               # 448 fp32 -> one PSUM bank
NCHUNK = CO // 128           # co chunks of 128 partitions

WDTYPE = "float16"           # "float16" | "float32r"

_cache = {}


def _build(reps=1, wdtype=WDTYPE, internal_io=False):
    """Build + compile the SPMD program.

    reps>1 wraps the whole load+compute+store body in a For_i hardware
    loop (used for timing). internal_io keeps x/y in device DRAM with a
    small token output (timing-only: no host transfer of the big tensors).
    """
    import contextlib

    import concourse.mybir as mybir
    import concourse.tile as tile
    from concourse import bacc

    mmdt = getattr(mybir.dt, wdtype)

    nc = bacc.Bacc("TRN2", target_bir_lowering=False, debug=False)
    if internal_io:
        xp_ap = nc.dram_tensor("xp_i", [BS, CI, HP, WP], mmdt).ap()
        y_ap = nc.dram_tensor("y_i", [BS, CO, H, W], mybir.dt.float32).ap()
        tok_ap = nc.dram_tensor(
            "tok", [128, NCHUNK], mybir.dt.float32, kind="ExternalOutput"
        ).ap()
    else:
        xp_ap = nc.dram_tensor(
            "xp", [BS, CI, HP, WP], mmdt, kind="ExternalInput"
        ).ap()
        y_ap = nc.dram_tensor(
            "y", [BS, CO, H, W], mybir.dt.float32, kind="ExternalOutput"
        ).ap()
        tok_ap = None
    wt_ap = nc.dram_tensor(
        "wt", [CI, NTAPS * CO], mmdt, kind="ExternalInput"
    ).ap()
    bt_ap = nc.dram_tensor(
        "bt", [128, NCHUNK], mybir.dt.float32, kind="ExternalInput"
    ).ap()

    cast_dma = wdtype == "float32r"  # fp32->fp32r rounding must happen on DMA

    with tile.TileContext(nc) as tc:
        with (
            tc.tile_pool(name="xw", bufs=1) as xw,
            tc.tile_pool(name="out", bufs=8) as outp,
            tc.tile_pool(name="ps", bufs=7, space="PSUM") as ps,
        ):
            wsb = xw.tile([CI, NTAPS * CO], mmdt, tag="w")
            bsb = xw.tile([128, NCHUNK], mybir.dt.float32, tag="b")
            if cast_dma:
                nc.gpsimd.dma_start(out=wsb[:], in_=wt_ap[:, :])
            else:
                # weights on the scalar engine's HWDGE queues: parallel
                # with the x loads below (gpsimd SWDGE queue)
                nc.scalar.dma_start(out=wsb[:], in_=wt_ap[:, :])
            nc.scalar.dma_start(out=bsb[:], in_=bt_ap[:, :])

            loop_cm = (
                tc.For_i(0, reps, 1, hint_engines=(mybir.EngineType.PE,))
                if reps > 1
                else contextlib.nullcontext()
            )
            with loop_cm:
                xsb = xw.tile([CI, BS * HP * WP], mmdt, tag="x")
                xdma = nc.gpsimd
                # priority chunk: rows 0..HB+1 of img0 — everything the first
                # matmul group reads — so PE can start ~1us in
                head = (HB + 2) * WP
                xflat0 = xp_ap[0].rearrange("c h w -> c (h w)")
                xdma.dma_start(out=xsb[:, 0:head], in_=xflat0[:, 0:head])
                xdma.dma_start(
                    out=xsb[:, head : HP * WP], in_=xflat0[:, head : HP * WP]
                )
                for img in range(1, BS):
                    xdma.dma_start(
                        out=xsb[:, img * HP * WP : (img + 1) * HP * WP],
                        in_=xp_ap[img].rearrange("c h w -> c (h w)")[:, :],
                    )
                xv = xsb[:].rearrange("c (n h w) -> c n h w", n=BS, h=HP)

                for c in range(NCHUNK):
                    for img in range(BS):
                        for hb in range(NB):
                            pt = ps.tile([128, NTILE], mybir.dt.float32, tag="acc")
                            for kh in range(KH):
                                for kw in range(KW):
                                    tap = kh * KW + kw
                                    r0 = hb * HB + kh
                                    nc.tensor.matmul(
                                        pt[:],
                                        wsb[
                                            :,
                                            tap * CO
                                            + c * 128 : tap * CO
                                            + (c + 1) * 128,
                                        ],
                                        xv[:, img, r0 : r0 + HB, kw : kw + W],
                                        start=(tap == 0),
                                        stop=(tap == NTAPS - 1),
                                    )
                            ot = outp.tile([128, NTILE], mybir.dt.float32, tag="o")
                            nc.scalar.activation(
                                ot[:],
                                pt[:],
                                mybir.ActivationFunctionType.Identity,
                                bias=bsb[:, c : c + 1],
                                scale=1.0,
                            )
                            nc.sync.dma_start(
                                out=y_ap[
                                    img,
                                    c * 128 : (c + 1) * 128,
                                    hb * HB : (hb + 1) * HB,
                                    :,
                                ],
                                in_=ot[:],
                            )
            if tok_ap is not None:
                nc.sync.dma_start(out=tok_ap[:, :], in_=bsb[:])
    nc.compile()
    return nc


def _get_nc(reps=1, wdtype=WDTYPE, internal_io=False):
    key = (reps, wdtype, internal_io)
    if key not in _cache:
        _cache[key] = _build(reps, wdtype, internal_io)
    return _cache[key]


def _prep_inputs(x, weight, bias, wdtype=WDTYPE):
    npdt = np.float16 if wdtype == "float16" else np.float32
    x = np.asarray(x)
    weight = np.ascontiguousarray(weight, dtype=np.float32)
    bias = np.ascontiguousarray(bias, dtype=np.float32)
    # fused pad+cast: one pass over x instead of pad(fp32) then astype
    xpad = np.zeros((B, CI, HP, WP), dtype=npdt)
    xpad[:, :, 1 : H + 1, 1 : W + 1] = x
    # [co, ci, kh, kw] -> [ci, (kh kw), co] flattened to [ci, 9*co]
    wt = np.ascontiguousarray(
        weight.transpose(1, 2, 3, 0).reshape(CI, NTAPS * CO).astype(npdt)
    )
    bt = np.ascontiguousarray(bias.reshape(NCHUNK, 128).T)
    in_maps = [
        {
            "xp": np.ascontiguousarray(xpad[i * BS : (i + 1) * BS]),
            "wt": wt,
            "bt": bt,
        }
        for i in range(N_CORES)
    ]
    return in_maps


def run_sharded(x, weight, bias, trace=False, reps=1, wdtype=WDTYPE):
    """Run on all 8 cores; returns (full_output, BassKernelResults)."""
    from concourse.bass_utils import run_bass_kernel_spmd

    nc = _get_nc(reps, wdtype)
    in_maps = _prep_inputs(x, weight, bias, wdtype)
    res = run_bass_kernel_spmd(nc, in_maps, list(range(N_CORES)), trace=trace)
    y = np.concatenate([res.results[i]["y"] for i in range(N_CORES)], axis=0)
    return y, res


def kernel(x, weight, bias):
    y, _ = run_sharded(x, weight, bias)
    return y

